# revision 10
# baseline (speedup 1.0000x reference)
"""Trainium2 Bass kernel for nn_CGT_21354577396059 (GPS-style GNN, 3 streams x 3 layers).

Strategy (8 NeuronCores, SPMD):
- Node-shard: core c owns nodes [2048c, 2048c+2048) = 8 graphs of 256 nodes.
- Activations feature-major in SBUF: hT [C=128 partitions, 2048 nodes] fp32,
  bf16 copies as matmul inputs.
- GIN segment_sum: edges dst-sorted per core into 64 windows of 32 dsts,
  padded per window to cap*128 rows; src rows gathered from a bf16
  node-major DRAM table (gpsimd dma_gather, 256B rows, 2048 idx/call);
  scatter via one-hot matmuls aggT[:, win] += gath_chunk.T @ onehot_chunk
  accumulated 16 windows (512 dsts) per PSUM bank.
- The bf16 node table is rebuilt each layer via PE transpose + 8-core
  AllGather (Shared output). Physical table rows are permuted so the
  staging write is a contiguous per-partition DMA; host maps src indices.
- Attention is graph-local: kT/qT projected once; scoresT per
  (head, graph, key-chunk) contract only that head's 32 partitions;
  exp on ACT; softmax sums via col-tiled ones-matmuls; o via col-tiled
  matmuls contracted over keys; normalization with DVE reciprocal + mul.

kernel(**inputs) takes the FULL unsharded inputs and returns
(pool(h0), pool(ha), pool(hb)) - tuple of [64, 128] float32 - like the reference.
"""
import sys
import numpy as np
import ml_dtypes

if "/opt/trn_rl_repo" not in sys.path:
    sys.path.insert(0, "/opt/trn_rl_repo")

import concourse.bass as bass  # noqa: F401
import concourse.tile as tile
from concourse import bacc, mybir
from concourse.bass_utils import run_bass_kernel_spmd

BF = ml_dtypes.bfloat16

# Problem constants (self-contained; no reads of /root/problem/*)
N_NODES = 16384
N_GRAPHS = 64
NPG = 256
FEA_DIM = 32
PE_DIM = 20
C = 128
HEADS = 4
HD = C // HEADS
L = 3
BN_EPS = 1e-5
S_BN = float(1.0 / np.sqrt(1.0 + BN_EPS))

N_CORES = 8
NPC = N_NODES // N_CORES   # 2048
NG_C = NPC // NPG          # 8 graphs per core
WIN = 32                   # dsts per scatter window
NWIN = NPC // WIN          # 64
GRP = 16                   # windows per PSUM group (512 dsts)
NGRP = NWIN // GRP         # 4
NCHUNK = NPC // 128        # 16
CALLS_PER_GRP = 2          # dma_gather calls per scatter group

fdt = mybir.dt.float32
bdt = mybir.dt.bfloat16
i16 = mybir.dt.int16
AF = mybir.ActivationFunctionType
AX = mybir.AxisListType
ALU = mybir.AluOpType
AG_GROUPS = [list(range(N_CORES))]


# ---------------------------------------------------------------------------
# Host-side data prep
# ---------------------------------------------------------------------------

def _phys_row(src):
    """Physical row of node `src` in the AllGathered table (see emit_table)."""
    core = src // NPC
    loc = src % NPC
    return core * NPC + (loc % 128) * NCHUNK + loc // 128


def _wrap_idxs(idx):
    """dma_gather idx layout [128, n/16] int16: idx i at (i%16, i//16),
    replicated across the 8 16-partition blocks."""
    n = len(idx)
    a = np.asarray(idx, np.int16).reshape(n // 16, 16).T
    return np.ascontiguousarray(np.tile(a, (8, 1)))


def _prep_edges_stream(edge_index):
    """Returns (cap, [(gidx_wrapped, onehot_pkc)] per core).

    cap = chunks (of 128 edge rows) per 32-dst window, uniform.
    onehot_pkc: [128, NWIN*cap, WIN] bf16 with onehot_pkc[p, t, :] the
    one-hot row of padded edge t*128+p (contiguous per-partition DMA).
    """
    src = np.asarray(edge_index[0]).astype(np.int64)
    dst = np.asarray(edge_index[1]).astype(np.int64)
    per_core_wins = []
    max_w = 0
    for c in range(N_CORES):
        m = (dst >= c * NPC) & (dst < (c + 1) * NPC)
        s, d = src[m], dst[m] - c * NPC
        order = np.argsort(d, kind="stable")
        s, d = s[order], d[order]
        wins = []
        for w in range(NWIN):
            mw = (d >= w * WIN) & (d < (w + 1) * WIN)
            wins.append((s[mw], d[mw] - w * WIN))
            max_w = max(max_w, int(mw.sum()))
        per_core_wins.append(wins)
    cap = (max_w + 127) // 128               # chunks per window
    cap_e = cap * 128
    out = []
    for c in range(N_CORES):
        srcs = np.zeros(NWIN * cap_e, np.int64)
        oh = np.zeros((NWIN * cap_e, WIN), BF)
        for w in range(NWIN):
            s, dloc = per_core_wins[c][w]
            n = len(s)
            srcs[w * cap_e:w * cap_e + n] = s
            oh[w * cap_e + np.arange(n), dloc] = 1
        phys = _phys_row(srcs)
        oh_pkc = np.ascontiguousarray(
            oh.reshape(NWIN * cap, 128, WIN).transpose(1, 0, 2))
        out.append((_wrap_idxs(phys), oh_pkc))
    return cap, out


def _pack_host(inputs):
    inp = {k: np.asarray(v) for k, v in inputs.items()}
    rt2 = 1.0 / np.sqrt(HD)

    blocks, offs = [], {}

    def add(name, arr):
        arr = np.asarray(arr, np.float32)
        k, m = arr.shape
        buf = np.zeros((128, m), BF)
        buf[:k] = arr.astype(BF)
        offs[name] = sum(b.shape[1] for b in blocks)
        blocks.append(buf)

    add("emb", inp["node_emb_w"])
    add("pe", inp["pe_lin_w"])
    add("I2", 2.0 * np.eye(C))       # h2 fold: ACT scale s gives 2s*h
    add("I1", np.eye(C))             # transpose identity + acc fold
    add("ones32", np.ones((C, HD)))
    for l in range(L):
        aw = inp["attn_in_w"][l]
        add(f"gw1_{l}", inp["gin_w1"][l])
        add(f"gw2_{l}", inp["gin_w2"][l])
        add(f"qT_{l}", (aw[0:C] * rt2).T)
        add(f"kT_{l}", aw[C:2 * C].T)
        add(f"vT_{l}", aw[2 * C:3 * C].T)
        add(f"ow_{l}", inp["attn_out_w"][l].T)
        add(f"m1_{l}", inp["mlp_w1"][l])
        add(f"m2a_{l}", inp["mlp_w2"][l][0:C])
        add(f"m2b_{l}", inp["mlp_w2"][l][C:2 * C])
    wts = np.ascontiguousarray(np.concatenate(blocks, axis=1))

    bvecs, boffs = [], {}

    def addb(name, vec):
        vec = np.asarray(vec, np.float32).reshape(-1)
        assert vec.shape == (C,)
        boffs[name] = len(bvecs)
        bvecs.append(vec)

    addb("eb", inp["node_emb_b"] + inp["pe_lin_b"])
    for l in range(L):
        ab = inp["attn_in_b"][l]
        addb(f"gb1_{l}", inp["gin_b1"][l])
        addb(f"sgb2_{l}", S_BN * inp["gin_b2"][l])
        addb(f"qb_{l}", ab[0:C] * rt2)
        addb(f"kb_{l}", ab[C:2 * C])
        addb(f"sob_{l}", S_BN * inp["attn_out_b"][l])
        addb(f"m1ba_{l}", inp["mlp_b1"][l][0:C])
        addb(f"m1bb_{l}", inp["mlp_b1"][l][C:2 * C])
        addb(f"smb2_{l}", S_BN * inp["mlp_b2"][l])
    biases = np.ascontiguousarray(np.stack(bvecs, axis=1).astype(np.float32))

    vbias = np.ascontiguousarray(np.stack(
        [np.tile(inp["attn_in_b"][l][2 * C:3 * C], (128, 1)) for l in range(L)]
    ).astype(np.float32))

    caps, edges = [], []
    for key in ("edge_index", "edge_index1", "edge_index2"):
        cap, per_core = _prep_edges_stream(inp[key])
        caps.append(cap)
        edges.append(per_core)

    xs = [inp["x"], inp["x1"], inp["x2"]]
    in_maps = []
    for c in range(N_CORES):
        m = {"wts": wts, "biases": biases, "vbias": vbias}
        sl = slice(c * NPC, (c + 1) * NPC)
        for s in range(3):
            m[f"xT{s}"] = np.ascontiguousarray(xs[s][sl].T.astype(BF))
            m[f"gidx{s}"] = edges[s][c][0]
            m[f"onehot{s}"] = edges[s][c][1]
        m["peT"] = np.ascontiguousarray(inp["pe"][sl].T.astype(BF))
        in_maps.append(m)

    return in_maps, caps, offs, boffs, wts.shape[1], biases.shape[1]


# ---------------------------------------------------------------------------
# Kernel builder
# ---------------------------------------------------------------------------


def _mm(nc, out, lhsT, rhs, start, stop, nmax=512):
    """matmul with moving free dim split to <=512 (ISA limit)."""
    n = rhs.shape[-1]
    assert out.shape[-1] == n
    for i in range(0, n, nmax):
        j = min(i + nmax, n)
        nc.tensor.matmul(out[:, i:j], lhsT, rhs[:, i:j], start=start, stop=stop)


def build_kernel(caps, offs, boffs, wcols, bcols, n_layers=L, n_streams=3,
                 dbg_stream=None):
    nc = bacc.Bacc("TRN2", target_bir_lowering=False, num_devices=N_CORES,
                   num_swdge_queues=4)

    wts_d = nc.dram_tensor("wts", [128, wcols], bdt, kind="ExternalInput")
    bias_d = nc.dram_tensor("biases", [128, bcols], fdt, kind="ExternalInput")
    vbias_d = nc.dram_tensor("vbias", [L, 128, 128], fdt, kind="ExternalInput")
    peT_d = nc.dram_tensor("peT", [PE_DIM, NPC], bdt, kind="ExternalInput")
    xT_d, gidx_d, oh_d = [], [], []
    for s in range(3):
        cap = caps[s]
        xT_d.append(nc.dram_tensor(f"xT{s}", [FEA_DIM, NPC], bdt,
                                   kind="ExternalInput"))
        gidx_d.append(nc.dram_tensor(f"gidx{s}", [128, NWIN * cap * 8], i16,
                                     kind="ExternalInput"))
        oh_d.append(nc.dram_tensor(f"onehot{s}", [128, NWIN * cap, WIN], bdt,
                                   kind="ExternalInput"))
    pool_out = nc.dram_tensor("pool_out", [3, C, NG_C], fdt,
                              kind="ExternalOutput")
    dbg_out = None
    if dbg_stream is not None:
        dbg_out = nc.dram_tensor("dbg_out", [C, NPC], fdt, kind="ExternalOutput")

    with tile.TileContext(nc) as tc:
        with (
            tc.tile_pool(name="const", bufs=1) as const_p,
            tc.tile_pool(name="hstate", bufs=1) as hstate_p,
            tc.tile_pool(name="big", bufs=1) as big_p,       # full-width tiles
            tc.tile_pool(name="chunk", bufs=2) as chunk_p,   # [C,1024]-ish tiles
            tc.tile_pool(name="gath", bufs=3) as gath_p,
            tc.tile_pool(name="ohp", bufs=2) as oh_p,
            tc.tile_pool(name="ps2", bufs=2, space="PSUM") as ps2,   # 2-bank
            tc.tile_pool(name="ps1", bufs=2, space="PSUM") as ps1,   # 1-bank
            tc.tile_pool(name="psS", bufs=2, space="PSUM") as psS,   # 1-bank
            tc.tile_pool(name="dram", bufs=4, space="DRAM") as dram_p,
        ):
            wts = const_p.tile([128, wcols], bdt)
            nc.sync.dma_start(wts[:], wts_d[:])
            bia = const_p.tile([128, bcols], fdt)
            nc.sync.dma_start(bia[:], bias_d[:])
            vbia = const_p.tile([128, L, 128], fdt)
            nc.sync.dma_start(vbia[:], vbias_d[:].rearrange("l p c -> p l c"))

            def W(name, width=128, rows=128):
                return wts[0:rows, offs[name]:offs[name] + width]

            def B(name):
                return bia[:, boffs[name]:boffs[name] + 1]

            hT = [hstate_p.tile([C, NPC], fdt, tag=f"hT{s}", name=f"hT{s}")
                  for s in range(3)]
            hB = [hstate_p.tile([C, NPC], bdt, tag=f"hB{s}", name=f"hB{s}")
                  for s in range(3)]
            gidx = []
            for s in range(n_streams):
                t = hstate_p.tile([128, NWIN * caps[s] * 8], i16,
                                  tag=f"gidx{s}", name=f"gidx{s}")
                nc.sync.dma_start(t[:], gidx_d[s][:])
                gidx.append(t)

            def emit_table(src_bf):
                """feature-major bf16 [C, NPC] -> node-major -> AllGather table.

                Staged node-major as [p, k, c] (node k*128+p at partition p,
                chunk k) so the DRAM write is contiguous per partition; host
                maps gather indices via _phys_row to match.
                """
                nm = big_p.tile([128, NCHUNK, C], bdt, tag="g_bf", name="nm")
                for k in range(NCHUNK):
                    pt = ps1.tile([128, 128], bdt, tag="ps1", name="pt")
                    nc.tensor.transpose(pt[:], src_bf[:, k * 128:(k + 1) * 128],
                                        W("I1"))
                    nc.vector.tensor_copy(nm[:, k, :], pt[:])
                agi = dram_p.tile([128, NCHUNK, C], bdt, tag="agi")
                nc.sync.dma_start(agi[:], nm[:])
                tab = dram_p.tile([N_NODES, C], bdt, tag="tab", name="tab",
                                  addr_space="Shared")
                nc.gpsimd.collective_compute(
                    "AllGather", ALU.bypass, replica_groups=AG_GROUPS,
                    ins=[agi.opt()], outs=[tab.opt()])
                return tab

            # ---------------- embedding ----------------
            tabs = [None] * 3
            for s in range(n_streams):
                xT = chunk_p.tile([FEA_DIM, NPC], bdt, tag="xT")
                nc.sync.dma_start(xT[:], xT_d[s][:])
                peT = chunk_p.tile([PE_DIM, NPC], bdt, tag="peT")
                nc.sync.dma_start(peT[:], peT_d[:])
                for ch in range(2):
                    sl = slice(ch * 1024, (ch + 1) * 1024)
                    ep = ps2.tile([C, 1024], fdt, tag="ps2")
                    _mm(nc, ep, W("emb", rows=FEA_DIM), xT[:, sl],
                        start=True, stop=False)
                    _mm(nc, ep, W("pe", rows=PE_DIM), peT[:, sl],
                        start=False, stop=True)
                    nc.scalar.activation(hT[s][:, sl], ep[:], AF.Identity,
                                         bias=B("eb"))
                    nc.vector.tensor_copy(hB[s][:, sl], hT[s][:, sl])
                tabs[s] = emit_table(hB[s])

            # ---------------- layers ----------------
            for l in range(n_layers):
                for s in range(n_streams):
                    _gps_layer(nc, l, caps[s], hT[s], hB[s], tabs, s,
                               gidx[s], oh_d[s], W, B, vbia,
                               big_p, chunk_p, gath_p, oh_p, ps2, ps1, psS,
                               emit_table, last=(l == n_layers - 1))

            # ---------------- pooling ----------------
            for s in range(n_streams):
                po = chunk_p.tile([C, NG_C], fdt, tag="pool")
                nc.vector.reduce_sum(
                    out=po[:],
                    in_=hT[s][:].rearrange("c (g n) -> c g n", g=NG_C),
                    axis=AX.X)
                nc.sync.dma_start(pool_out[s], po[:])

            if dbg_stream is not None:
                nc.sync.dma_start(dbg_out[:], hT[dbg_stream][:])

    nc.compile()
    return nc


def _gps_layer(nc, l, cap, hT, hB, tabs, s, gidx, oh_d, W, B, vbia,
               big_p, chunk_p, gath_p, oh_p, ps2, ps1, psS, emit_table, last):
    AFI = AF.Identity

    # ---------------- GIN (windowed one-hot scatter) ----------------
    # Emitted before attention so the Pool-engine gathers for the next
    # stream run while the PE does this stream's attention.
    g_bf = big_p.tile([C, NPC], bdt, tag="g_bf", name="g_bf")
    tab = tabs[s]
    gpc = GRP * cap                         # chunks per group
    gcall = gpc * 128 // CALLS_PER_GRP      # gather rows per call
    chunks_per_call = gpc // CALLS_PER_GRP
    qn = 0
    for gr in range(NGRP):
        base_c = gr * gpc                   # first chunk of group
        gts = []
        for cc in range(CALLS_PER_GRP):
            gt = gath_p.tile([128, chunks_per_call, C], bdt, tag="gath",
                             name="gt")
            c0 = base_c + cc * chunks_per_call
            isl = gidx[:, c0 * 8:(c0 + chunks_per_call) * 8]
            nc.gpsimd.dma_gather(gt[:], tab[:], isl, gcall, gcall, C,
                                 queue_num=qn, single_packet=False)
            qn = (qn + 1) % 4
            gts.append(gt)
        ohs = oh_p.tile([128, gpc, WIN], bdt, tag="ohs", name="ohs")
        nc.sync.dma_start(ohs[:], oh_d[:, base_c:base_c + gpc, :])
        ps = psS.tile([C, GRP * WIN], fdt, tag="psS", name="psc")
        for wl in range(GRP):
            for t in range(cap):
                tci = wl * cap + t
                gt = gts[tci // chunks_per_call]
                nc.tensor.matmul(ps[:, wl * WIN:(wl + 1) * WIN],
                                 gt[:, tci % chunks_per_call, :],
                                 ohs[:, tci, :],
                                 start=(t == 0), stop=(t == cap - 1))
        slg = slice(gr * GRP * WIN, (gr + 1) * GRP * WIN)
        nc.vector.tensor_add(g_bf[:, slg], ps[:], hT[:, slg])

    # ---------------- attention (graph-local) ----------------
    # q/k feature-major, heads split in two 64-partition banks so head
    # slices sit at partition bases {0, 32} (matmul operand base limit).
    qT = big_p.tile([64, 2, NPC], bdt, tag="qT", name="qT")
    kT = big_p.tile([64, 2, NPC], bdt, tag="kT", name="kT")
    for ch in range(2):
        sl = slice(ch * 1024, (ch + 1) * 1024)
        qp = ps2.tile([C, 1024], fdt, tag="ps2")
        _mm(nc, qp, W(f"qT_{l}"), hB[:, sl], start=True, stop=True)
        nc.scalar.activation(qT[0:64, 0, sl], qp[0:64, :], AFI,
                             bias=B(f"qb_{l}")[0:64, :])
        nc.scalar.activation(qT[0:64, 1, sl], qp[64:128, :], AFI,
                             bias=B(f"qb_{l}")[64:128, :])
        kp = ps2.tile([C, 1024], fdt, tag="ps2")
        _mm(nc, kp, W(f"kT_{l}"), hB[:, sl], start=True, stop=True)
        nc.scalar.activation(kT[0:64, 0, sl], kp[0:64, :], AFI,
                             bias=B(f"kb_{l}")[0:64, :])
        nc.scalar.activation(kT[0:64, 1, sl], kp[64:128, :], AFI,
                             bias=B(f"kb_{l}")[64:128, :])
    v_sb = big_p.tile([128, NCHUNK, C], bdt, tag="v_sb", name="v_sb")
    for k in range(NCHUNK):
        vp = ps1.tile([128, C], fdt, tag="ps1")
        nc.tensor.matmul(vp[:], hB[:, k * 128:(k + 1) * 128], W(f"vT_{l}"),
                         start=True, stop=True)
        nc.vector.tensor_add(v_sb[:, k, :], vp[:], vbia[:, l, :])

    o_bf = big_p.tile([C, NPC], bdt, tag="o_bf", name="o_bf")
    recip = big_p.tile([128, NPC], bdt, tag="recip", name="recip")
    for gh in range(2):           # half = 4 graphs = 1024 nodes
        expT = big_p.tile([128, 2, HEADS, 4, NPG], bdt, tag="expT",
                          name="expT")
        for h in range(HEADS):
            hsl = slice(32 * (h % 2), 32 * (h % 2) + 32)
            hb = h // 2
            for kc in range(2):
                sp = ps2.tile([128, 4, NPG], fdt, tag="ps2")
                for gi in range(4):
                    g = gh * 4 + gi
                    ksl = kT[hsl, hb,
                             g * NPG + kc * 128:g * NPG + kc * 128 + 128]
                    qsl = qT[hsl, hb, g * NPG:(g + 1) * NPG]
                    nc.tensor.matmul(sp[:, gi, :], ksl, qsl,
                                     start=True, stop=True)
                nc.scalar.activation(expT[:, kc, h, :, :], sp[:], AF.Exp)
        sm = ps2.tile([128, 1024], fdt, tag="ps2")
        for h in range(HEADS):
            for qc in range(2):
                osl = sm[32 * h:32 * h + 32, qc * 512:(qc + 1) * 512]
                for kc in range(2):
                    rhs = expT[:, kc, h, :, :].rearrange("p g q -> p (g q)")
                    nc.tensor.matmul(
                        osl, W("ones32", width=HD),
                        rhs[:, qc * 512:(qc + 1) * 512],
                        start=(kc == 0), stop=(kc == 1),
                        tile_position=(0, 32 * h))
        with nc.allow_low_precision(reason="softmax recip bf16 ok at 2e-2 gate"):
            nc.vector.reciprocal(recip[:, gh * 1024:(gh + 1) * 1024], sm[:])
        for gi in range(4):
            g = gh * 4 + gi
            op = ps1.tile([128, NPG], fdt, tag="ps1")
            for h in range(HEADS):
                for kc in range(2):
                    lhs = v_sb[:, g * 2 + kc, 32 * h:32 * h + 32]
                    rhs = expT[:, kc, h, gi, :]
                    nc.tensor.matmul(op[32 * h:32 * h + 32, :], lhs, rhs,
                                     start=(kc == 0), stop=(kc == 1),
                                     tile_position=(0, 32 * h))
            nc.vector.tensor_mul(o_bf[:, g * NPG:(g + 1) * NPG], op[:],
                                 recip[:, g * NPG:(g + 1) * NPG])

    # GIN MLP + combine with attention: acc = h1 + h2
    acc_bf = big_p.tile([C, NPC], bdt, tag="acc_bf")
    r_bf = big_p.tile([C, NPC], bdt, tag="r_bf")
    for ch in range(2):
        sl = slice(ch * 1024, (ch + 1) * 1024)
        tp = ps2.tile([C, 1024], fdt, tag="ps2")
        _mm(nc, tp, W(f"gw1_{l}"), g_bf[:, sl], start=True, stop=True)
        nc.scalar.activation(r_bf[:, sl], tp[:], AF.Relu, bias=B(f"gb1_{l}"))
        up = ps2.tile([C, 1024], fdt, tag="ps2")
        _mm(nc, up, W(f"gw2_{l}"), r_bf[:, sl], start=True, stop=True)
        h1 = chunk_p.tile([C, 1024], bdt, tag="h1")
        nc.scalar.activation(h1[:], up[:], AFI, bias=B(f"sgb2_{l}"), scale=S_BN)
        ap2 = ps2.tile([C, 1024], fdt, tag="ps2")
        _mm(nc, ap2, W(f"ow_{l}"), o_bf[:, sl], start=True, stop=False)
        _mm(nc, ap2, W("I2"), hB[:, sl], start=False, stop=True)
        h2 = chunk_p.tile([C, 1024], bdt, tag="h2")
        nc.scalar.activation(h2[:], ap2[:], AFI, bias=B(f"sob_{l}"), scale=S_BN)
        nc.vector.tensor_add(acc_bf[:, sl], h1[:], h2[:])

    # MLP (hidden 256 split in two bf16 halves r2a/r2b, then one accumulate)
    for ch in range(2):
        sl = slice(ch * 1024, (ch + 1) * 1024)
        r2 = []
        for mh in range(2):
            mp = ps2.tile([C, 1024], fdt, tag="ps2")
            _mm(nc, mp, W(f"m1_{l}", width=256)[:, mh * 128:(mh + 1) * 128],
                acc_bf[:, sl], start=True, stop=True)
            r2h = chunk_p.tile([C, 1024], bdt, tag=f"r2{mh}")
            bname = f"m1ba_{l}" if mh == 0 else f"m1bb_{l}"
            nc.scalar.activation(r2h[:], mp[:], AF.Relu, bias=B(bname))
            r2.append(r2h)
        m2p = ps2.tile([C, 1024], fdt, tag="ps2")
        _mm(nc, m2p, W(f"m2a_{l}"), r2[0][:], start=True, stop=False)
        _mm(nc, m2p, W(f"m2b_{l}"), r2[1][:], start=False, stop=False)
        _mm(nc, m2p, W("I1"), acc_bf[:, sl], start=False, stop=True)
        dh = chunk_p.tile([C, 1024], fdt, tag="dh")
        nc.scalar.activation(dh[:], m2p[:], AFI, bias=B(f"smb2_{l}"), scale=S_BN)
        nc.vector.tensor_add(hT[:, sl], hT[:, sl], dh[:])
        nc.vector.tensor_copy(hB[:, sl], hT[:, sl])

    if not last:
        tabs[s] = emit_table(hB)


# ---------------------------------------------------------------------------
# Entry point
# ---------------------------------------------------------------------------

_CACHE = {}


def _get_kernel(caps, offs, boffs, wcols, bcols, **kw):
    key = (tuple(caps), wcols, bcols, tuple(sorted(kw.items())))
    if key not in _CACHE:
        _CACHE[key] = build_kernel(caps, offs, boffs, wcols, bcols, **kw)
    return _CACHE[key]


def kernel(**inputs):
    in_maps, caps, offs, boffs, wcols, bcols = _pack_host(inputs)
    nc = _get_kernel(caps, offs, boffs, wcols, bcols)
    res = run_bass_kernel_spmd(nc, in_maps, core_ids=list(range(N_CORES)))
    pools = []
    for si in range(3):
        parts = [np.asarray(res.results[c]["pool_out"][si])
                 for c in range(N_CORES)]
        full = np.concatenate(parts, axis=1)          # [C, 64]
        pools.append(np.ascontiguousarray(full.T).astype(np.float32))
    return tuple(pools)


# revision 13
# speedup vs baseline: 1.7503x; 1.7503x over previous
"""Trainium2 Bass kernel for nn_CGT_21354577396059 (GPS-style GNN, 3 streams x 3 layers).

Strategy (8 NeuronCores, SPMD):
- Node-shard: core c owns nodes [2048c, 2048c+2048) = 8 graphs of 256 nodes.
- Activations feature-major in SBUF: hT [C=128 partitions, 2048 nodes] fp32,
  bf16 copies as matmul inputs.
- GIN segment_sum: edges dst-sorted per core into 32 windows of 64 dsts,
  padded per window to cap*128 rows; src rows gathered from a bf16
  node-major DRAM table (gpsimd dma_gather, 256B rows, 1024 idx/call =
  the single-packet HW cap; multi-packet mode is ~5x slower per idx);
  scatter via one-hot matmuls aggT[:, win] += gath_chunk.T @ onehot_chunk
  accumulated 8 windows (512 dsts) per PSUM bank. GIN is emitted before
  attention so Pool-engine gathers overlap PE attention work.
- The bf16 node table is rebuilt each layer via PE transpose + 8-core
  AllGather (Local output: Shared-output AllGather measured 3.4x slower).
  Physical table rows are permuted so the staging write is a contiguous
  per-partition DMA; host maps src indices via _phys_row.
- Attention is graph-local: kT/qT projected once; scoresT per
  (head, graph, key-chunk) contract only that head's 32 partitions;
  exp on ACT; softmax sums via col-tiled ones-matmuls; o via col-tiled
  matmuls contracted over keys; normalization with DVE reciprocal + mul.

kernel(**inputs) takes the FULL unsharded inputs and returns
(pool(h0), pool(ha), pool(hb)) - tuple of [64, 128] float32 - like the reference.
"""
import sys
import numpy as np
import ml_dtypes

if "/opt/trn_rl_repo" not in sys.path:
    sys.path.insert(0, "/opt/trn_rl_repo")

import concourse.bass as bass  # noqa: F401
import concourse.tile as tile
from concourse import bacc, mybir
from concourse.bass_utils import run_bass_kernel_spmd

BF = ml_dtypes.bfloat16

# Problem constants (self-contained; no reads of /root/problem/*)
N_NODES = 16384
N_GRAPHS = 64
NPG = 256
FEA_DIM = 32
PE_DIM = 20
C = 128
HEADS = 4
HD = C // HEADS
L = 3
BN_EPS = 1e-5
S_BN = float(1.0 / np.sqrt(1.0 + BN_EPS))

N_CORES = 8
NPC = N_NODES // N_CORES   # 2048
NG_C = NPC // NPG          # 8 graphs per core
WIN = 64                   # dsts per scatter window
NWIN = NPC // WIN          # 32
GRP = 8                    # windows per PSUM group (512 dsts)
NGRP = NWIN // GRP         # 4
NCHUNK = NPC // 128        # 16
GCALL = 1024               # rows per dma_gather call (single-packet HW cap)

fdt = mybir.dt.float32
bdt = mybir.dt.bfloat16
i16 = mybir.dt.int16
AF = mybir.ActivationFunctionType
AX = mybir.AxisListType
ALU = mybir.AluOpType
AG_GROUPS = [list(range(N_CORES))]


# ---------------------------------------------------------------------------
# Host-side data prep
# ---------------------------------------------------------------------------

def _phys_row(src):
    """Physical row of node `src` in the AllGathered table (see emit_table)."""
    core = src // NPC
    loc = src % NPC
    return core * NPC + (loc % 128) * NCHUNK + loc // 128


def _wrap_idxs(idx):
    """dma_gather idx layout [128, n/16] int16: idx i at (i%16, i//16),
    replicated across the 8 16-partition blocks."""
    n = len(idx)
    a = np.asarray(idx, np.int16).reshape(n // 16, 16).T
    return np.ascontiguousarray(np.tile(a, (8, 1)))


def _prep_edges_stream(edge_index):
    """Returns (cap, [(gidx_wrapped, onehot_pkc)] per core).

    cap = chunks (of 128 edge rows) per 32-dst window, uniform.
    onehot_pkc: [128, NWIN*cap, WIN] bf16 with onehot_pkc[p, t, :] the
    one-hot row of padded edge t*128+p (contiguous per-partition DMA).
    """
    src = np.asarray(edge_index[0]).astype(np.int64)
    dst = np.asarray(edge_index[1]).astype(np.int64)
    per_core_wins = []
    max_w = 0
    for c in range(N_CORES):
        m = (dst >= c * NPC) & (dst < (c + 1) * NPC)
        s, d = src[m], dst[m] - c * NPC
        order = np.argsort(d, kind="stable")
        s, d = s[order], d[order]
        wins = []
        for w in range(NWIN):
            mw = (d >= w * WIN) & (d < (w + 1) * WIN)
            wins.append((s[mw], d[mw] - w * WIN))
            max_w = max(max_w, int(mw.sum()))
        per_core_wins.append(wins)
    cap = (max_w + 127) // 128               # chunks per window
    cap_e = cap * 128
    out = []
    for c in range(N_CORES):
        srcs = np.zeros(NWIN * cap_e, np.int64)
        oh = np.zeros((NWIN * cap_e, WIN), BF)
        for w in range(NWIN):
            s, dloc = per_core_wins[c][w]
            n = len(s)
            srcs[w * cap_e:w * cap_e + n] = s
            oh[w * cap_e + np.arange(n), dloc] = 1
        phys = _phys_row(srcs)
        oh_pkc = np.ascontiguousarray(
            oh.reshape(NWIN * cap, 128, WIN).transpose(1, 0, 2))
        out.append((_wrap_idxs(phys), oh_pkc))
    return cap, out


def _pack_host(inputs):
    inp = {k: np.asarray(v) for k, v in inputs.items()}
    rt2 = 1.0 / np.sqrt(HD)

    blocks, offs = [], {}

    def add(name, arr):
        arr = np.asarray(arr, np.float32)
        k, m = arr.shape
        buf = np.zeros((128, m), BF)
        buf[:k] = arr.astype(BF)
        offs[name] = sum(b.shape[1] for b in blocks)
        blocks.append(buf)

    add("emb", inp["node_emb_w"])
    add("pe", inp["pe_lin_w"])
    add("I2", 2.0 * np.eye(C))       # h2 fold: ACT scale s gives 2s*h
    add("I1", np.eye(C))             # transpose identity + acc fold
    add("ones32", np.ones((C, HD)))
    for l in range(L):
        aw = inp["attn_in_w"][l]
        add(f"gw1_{l}", inp["gin_w1"][l])
        add(f"gw2_{l}", inp["gin_w2"][l])
        add(f"qT_{l}", (aw[0:C] * rt2).T)
        add(f"kT_{l}", aw[C:2 * C].T)
        add(f"vT_{l}", aw[2 * C:3 * C].T)
        add(f"ow_{l}", inp["attn_out_w"][l].T)
        add(f"m1_{l}", inp["mlp_w1"][l])
        add(f"m2a_{l}", inp["mlp_w2"][l][0:C])
        add(f"m2b_{l}", inp["mlp_w2"][l][C:2 * C])
    wts = np.ascontiguousarray(np.concatenate(blocks, axis=1))

    bvecs, boffs = [], {}

    def addb(name, vec):
        vec = np.asarray(vec, np.float32).reshape(-1)
        assert vec.shape == (C,)
        boffs[name] = len(bvecs)
        bvecs.append(vec)

    addb("eb", inp["node_emb_b"] + inp["pe_lin_b"])
    for l in range(L):
        ab = inp["attn_in_b"][l]
        addb(f"gb1_{l}", inp["gin_b1"][l])
        addb(f"sgb2_{l}", S_BN * inp["gin_b2"][l])
        addb(f"qb_{l}", ab[0:C] * rt2)
        addb(f"kb_{l}", ab[C:2 * C])
        addb(f"sob_{l}", S_BN * inp["attn_out_b"][l])
        addb(f"m1ba_{l}", inp["mlp_b1"][l][0:C])
        addb(f"m1bb_{l}", inp["mlp_b1"][l][C:2 * C])
        addb(f"smb2_{l}", S_BN * inp["mlp_b2"][l])
    biases = np.ascontiguousarray(np.stack(bvecs, axis=1).astype(np.float32))

    vbias = np.ascontiguousarray(np.stack(
        [np.tile(inp["attn_in_b"][l][2 * C:3 * C], (128, 1)) for l in range(L)]
    ).astype(np.float32))

    caps, edges = [], []
    for key in ("edge_index", "edge_index1", "edge_index2"):
        cap, per_core = _prep_edges_stream(inp[key])
        caps.append(cap)
        edges.append(per_core)

    xs = [inp["x"], inp["x1"], inp["x2"]]
    in_maps = []
    for c in range(N_CORES):
        m = {"wts": wts, "biases": biases, "vbias": vbias}
        sl = slice(c * NPC, (c + 1) * NPC)
        for s in range(3):
            m[f"xT{s}"] = np.ascontiguousarray(xs[s][sl].T.astype(BF))
            m[f"gidx{s}"] = edges[s][c][0]
            m[f"onehot{s}"] = edges[s][c][1]
        m["peT"] = np.ascontiguousarray(inp["pe"][sl].T.astype(BF))
        in_maps.append(m)

    return in_maps, caps, offs, boffs, wts.shape[1], biases.shape[1]


# ---------------------------------------------------------------------------
# Kernel builder
# ---------------------------------------------------------------------------


def _mm(nc, out, lhsT, rhs, start, stop, nmax=512):
    """matmul with moving free dim split to <=512 (ISA limit)."""
    n = rhs.shape[-1]
    assert out.shape[-1] == n
    for i in range(0, n, nmax):
        j = min(i + nmax, n)
        nc.tensor.matmul(out[:, i:j], lhsT, rhs[:, i:j], start=start, stop=stop)


def build_kernel(caps, offs, boffs, wcols, bcols, n_layers=L, n_streams=3,
                 dbg_stream=None):
    nc = bacc.Bacc("TRN2", target_bir_lowering=False, num_devices=N_CORES,
                   num_swdge_queues=4)

    wts_d = nc.dram_tensor("wts", [128, wcols], bdt, kind="ExternalInput")
    bias_d = nc.dram_tensor("biases", [128, bcols], fdt, kind="ExternalInput")
    vbias_d = nc.dram_tensor("vbias", [L, 128, 128], fdt, kind="ExternalInput")
    peT_d = nc.dram_tensor("peT", [PE_DIM, NPC], bdt, kind="ExternalInput")
    xT_d, gidx_d, oh_d = [], [], []
    for s in range(3):
        cap = caps[s]
        xT_d.append(nc.dram_tensor(f"xT{s}", [FEA_DIM, NPC], bdt,
                                   kind="ExternalInput"))
        gidx_d.append(nc.dram_tensor(f"gidx{s}", [128, NWIN * cap * 8], i16,
                                     kind="ExternalInput"))
        oh_d.append(nc.dram_tensor(f"onehot{s}", [128, NWIN * cap, WIN], bdt,
                                   kind="ExternalInput"))
    pool_out = nc.dram_tensor("pool_out", [3, C, NG_C], fdt,
                              kind="ExternalOutput")
    dbg_out = None
    if dbg_stream is not None:
        dbg_out = nc.dram_tensor("dbg_out", [C, NPC], fdt, kind="ExternalOutput")

    with tile.TileContext(nc) as tc:
        with (
            tc.tile_pool(name="const", bufs=1) as const_p,
            tc.tile_pool(name="hstate", bufs=1) as hstate_p,
            tc.tile_pool(name="big", bufs=1) as big_p,       # full-width tiles
            tc.tile_pool(name="chunk", bufs=2) as chunk_p,   # [C,1024]-ish tiles
            tc.tile_pool(name="gath", bufs=16) as gath_p,
            tc.tile_pool(name="ohp", bufs=2) as oh_p,
            tc.tile_pool(name="ps2", bufs=2, space="PSUM") as ps2,   # 2-bank
            tc.tile_pool(name="ps1", bufs=2, space="PSUM") as ps1,   # 1-bank
            tc.tile_pool(name="psS", bufs=2, space="PSUM") as psS,   # 1-bank
            tc.tile_pool(name="dram", bufs=4, space="DRAM") as dram_p,
        ):
            wts = const_p.tile([128, wcols], bdt)
            nc.sync.dma_start(wts[:], wts_d[:])
            bia = const_p.tile([128, bcols], fdt)
            nc.sync.dma_start(bia[:], bias_d[:])
            vbia = const_p.tile([128, L, 128], fdt)
            nc.sync.dma_start(vbia[:], vbias_d[:].rearrange("l p c -> p l c"))

            def W(name, width=128, rows=128):
                return wts[0:rows, offs[name]:offs[name] + width]

            def B(name):
                return bia[:, boffs[name]:boffs[name] + 1]

            hT = [hstate_p.tile([C, NPC], fdt, tag=f"hT{s}", name=f"hT{s}")
                  for s in range(3)]
            hB = [hstate_p.tile([C, NPC], bdt, tag=f"hB{s}", name=f"hB{s}")
                  for s in range(3)]
            gidx = []
            for s in range(n_streams):
                t = hstate_p.tile([128, NWIN * caps[s] * 8], i16,
                                  tag=f"gidx{s}", name=f"gidx{s}")
                nc.sync.dma_start(t[:], gidx_d[s][:])
                gidx.append(t)

            def emit_table(src_bf):
                """feature-major bf16 [C, NPC] -> node-major -> AllGather table.

                Staged node-major as [p, k, c] (node k*128+p at partition p,
                chunk k) so the DRAM write is contiguous per partition; host
                maps gather indices via _phys_row to match.
                """
                nm = big_p.tile([128, NCHUNK, C], bdt, tag="g_bf", name="nm")
                for k in range(NCHUNK):
                    pt = ps1.tile([128, 128], bdt, tag="ps1", name="pt")
                    nc.tensor.transpose(pt[:], src_bf[:, k * 128:(k + 1) * 128],
                                        W("I1"))
                    nc.vector.tensor_copy(nm[:, k, :], pt[:])
                agi = dram_p.tile([128, NCHUNK, C], bdt, tag="agi")
                nc.sync.dma_start(agi[:], nm[:])
                tab = dram_p.tile([N_NODES, C], bdt, tag="tab", name="tab")
                nc.gpsimd.collective_compute(
                    "AllGather", ALU.bypass, replica_groups=AG_GROUPS,
                    ins=[agi.opt()], outs=[tab.opt()])
                return tab

            # ---------------- embedding ----------------
            tabs = [None] * 3
            for s in range(n_streams):
                xT = chunk_p.tile([FEA_DIM, NPC], bdt, tag="xT")
                nc.sync.dma_start(xT[:], xT_d[s][:])
                peT = chunk_p.tile([PE_DIM, NPC], bdt, tag="peT")
                nc.sync.dma_start(peT[:], peT_d[:])
                for ch in range(2):
                    sl = slice(ch * 1024, (ch + 1) * 1024)
                    ep = ps2.tile([C, 1024], fdt, tag="ps2")
                    _mm(nc, ep, W("emb", rows=FEA_DIM), xT[:, sl],
                        start=True, stop=False)
                    _mm(nc, ep, W("pe", rows=PE_DIM), peT[:, sl],
                        start=False, stop=True)
                    nc.scalar.activation(hT[s][:, sl], ep[:], AF.Identity,
                                         bias=B("eb"))
                    nc.vector.tensor_copy(hB[s][:, sl], hT[s][:, sl])
                tabs[s] = emit_table(hB[s])

            # ---------------- layers ----------------
            for l in range(n_layers):
                for s in range(n_streams):
                    _gps_layer(nc, l, caps[s], hT[s], hB[s], tabs, s,
                               gidx[s], oh_d[s], W, B, vbia,
                               big_p, chunk_p, gath_p, oh_p, ps2, ps1, psS,
                               emit_table, last=(l == n_layers - 1))

            # ---------------- pooling ----------------
            for s in range(n_streams):
                po = chunk_p.tile([C, NG_C], fdt, tag="pool")
                nc.vector.reduce_sum(
                    out=po[:],
                    in_=hT[s][:].rearrange("c (g n) -> c g n", g=NG_C),
                    axis=AX.X)
                nc.sync.dma_start(pool_out[s], po[:])

            if dbg_stream is not None:
                nc.sync.dma_start(dbg_out[:], hT[dbg_stream][:])

    nc.compile()
    return nc


def _gps_layer(nc, l, cap, hT, hB, tabs, s, gidx, oh_d, W, B, vbia,
               big_p, chunk_p, gath_p, oh_p, ps2, ps1, psS, emit_table, last):
    AFI = AF.Identity

    # ---------------- GIN (windowed one-hot scatter) ----------------
    # Emitted before attention so the Pool-engine gathers for the next
    # stream run while the PE does this stream's attention.
    g_bf = big_p.tile([C, NPC], bdt, tag="g_bf", name="g_bf")
    tab = tabs[s]
    gpc = GRP * cap                         # chunks per group
    calls_per_grp = gpc * 128 // GCALL
    chunks_per_call = GCALL // 128
    assert gpc * 128 % GCALL == 0
    qn = 0
    for gr in range(NGRP):
        base_c = gr * gpc                   # first chunk of group
        gts = []
        for cc in range(calls_per_grp):
            gt = gath_p.tile([128, chunks_per_call, C], bdt, tag="gath",
                             name="gt")
            c0 = base_c + cc * chunks_per_call
            isl = gidx[:, c0 * 8:(c0 + chunks_per_call) * 8]
            nc.gpsimd.dma_gather(gt[:], tab[:], isl, GCALL, GCALL, C,
                                 queue_num=qn)
            qn = (qn + 1) % 4
            gts.append(gt)
        ohs = oh_p.tile([128, gpc, WIN], bdt, tag="ohs", name="ohs")
        nc.sync.dma_start(ohs[:], oh_d[:, base_c:base_c + gpc, :])
        ps = psS.tile([C, GRP * WIN], fdt, tag="psS", name="psc")
        for wl in range(GRP):
            for t in range(cap):
                tci = wl * cap + t
                gt = gts[tci // chunks_per_call]
                nc.tensor.matmul(ps[:, wl * WIN:(wl + 1) * WIN],
                                 gt[:, tci % chunks_per_call, :],
                                 ohs[:, tci, :],
                                 start=(t == 0), stop=(t == cap - 1))
        slg = slice(gr * GRP * WIN, (gr + 1) * GRP * WIN)
        nc.vector.tensor_add(g_bf[:, slg], ps[:], hT[:, slg])

    # ---------------- attention (graph-local) ----------------
    # q/k feature-major, heads split in two 64-partition banks so head
    # slices sit at partition bases {0, 32} (matmul operand base limit).
    qT = big_p.tile([64, 2, NPC], bdt, tag="qT", name="qT")
    kT = big_p.tile([64, 2, NPC], bdt, tag="kT", name="kT")
    for ch in range(2):
        sl = slice(ch * 1024, (ch + 1) * 1024)
        qp = ps2.tile([C, 1024], fdt, tag="ps2")
        _mm(nc, qp, W(f"qT_{l}"), hB[:, sl], start=True, stop=True)
        nc.scalar.activation(qT[0:64, 0, sl], qp[0:64, :], AFI,
                             bias=B(f"qb_{l}")[0:64, :])
        nc.scalar.activation(qT[0:64, 1, sl], qp[64:128, :], AFI,
                             bias=B(f"qb_{l}")[64:128, :])
        kp = ps2.tile([C, 1024], fdt, tag="ps2")
        _mm(nc, kp, W(f"kT_{l}"), hB[:, sl], start=True, stop=True)
        nc.scalar.activation(kT[0:64, 0, sl], kp[0:64, :], AFI,
                             bias=B(f"kb_{l}")[0:64, :])
        nc.scalar.activation(kT[0:64, 1, sl], kp[64:128, :], AFI,
                             bias=B(f"kb_{l}")[64:128, :])
    v_sb = big_p.tile([128, NCHUNK, C], bdt, tag="v_sb", name="v_sb")
    for k in range(NCHUNK):
        vp = ps1.tile([128, C], fdt, tag="ps1")
        nc.tensor.matmul(vp[:], hB[:, k * 128:(k + 1) * 128], W(f"vT_{l}"),
                         start=True, stop=True)
        nc.vector.tensor_add(v_sb[:, k, :], vp[:], vbia[:, l, :])

    o_bf = big_p.tile([C, NPC], bdt, tag="o_bf", name="o_bf")
    recip = big_p.tile([128, NPC], bdt, tag="recip", name="recip")
    for gh in range(2):           # half = 4 graphs = 1024 nodes
        expT = big_p.tile([128, 2, HEADS, 4, NPG], bdt, tag="expT",
                          name="expT")
        for h in range(HEADS):
            hsl = slice(32 * (h % 2), 32 * (h % 2) + 32)
            hb = h // 2
            for kc in range(2):
                sp = ps2.tile([128, 4, NPG], fdt, tag="ps2")
                for gi in range(4):
                    g = gh * 4 + gi
                    ksl = kT[hsl, hb,
                             g * NPG + kc * 128:g * NPG + kc * 128 + 128]
                    qsl = qT[hsl, hb, g * NPG:(g + 1) * NPG]
                    nc.tensor.matmul(sp[:, gi, :], ksl, qsl,
                                     start=True, stop=True)
                nc.scalar.activation(expT[:, kc, h, :, :], sp[:], AF.Exp)
        sm = ps2.tile([128, 1024], fdt, tag="ps2")
        for h in range(HEADS):
            for qc in range(2):
                osl = sm[32 * h:32 * h + 32, qc * 512:(qc + 1) * 512]
                for kc in range(2):
                    rhs = expT[:, kc, h, :, :].rearrange("p g q -> p (g q)")
                    nc.tensor.matmul(
                        osl, W("ones32", width=HD),
                        rhs[:, qc * 512:(qc + 1) * 512],
                        start=(kc == 0), stop=(kc == 1),
                        tile_position=(0, 32 * h))
        with nc.allow_low_precision(reason="softmax recip bf16 ok at 2e-2 gate"):
            nc.vector.reciprocal(recip[:, gh * 1024:(gh + 1) * 1024], sm[:])
        for gi in range(4):
            g = gh * 4 + gi
            op = ps1.tile([128, NPG], fdt, tag="ps1")
            for h in range(HEADS):
                for kc in range(2):
                    lhs = v_sb[:, g * 2 + kc, 32 * h:32 * h + 32]
                    rhs = expT[:, kc, h, gi, :]
                    nc.tensor.matmul(op[32 * h:32 * h + 32, :], lhs, rhs,
                                     start=(kc == 0), stop=(kc == 1),
                                     tile_position=(0, 32 * h))
            nc.vector.tensor_mul(o_bf[:, g * NPG:(g + 1) * NPG], op[:],
                                 recip[:, g * NPG:(g + 1) * NPG])

    # GIN MLP + combine with attention: acc = h1 + h2
    acc_bf = big_p.tile([C, NPC], bdt, tag="acc_bf")
    r_bf = big_p.tile([C, NPC], bdt, tag="r_bf")
    for ch in range(2):
        sl = slice(ch * 1024, (ch + 1) * 1024)
        tp = ps2.tile([C, 1024], fdt, tag="ps2")
        _mm(nc, tp, W(f"gw1_{l}"), g_bf[:, sl], start=True, stop=True)
        nc.scalar.activation(r_bf[:, sl], tp[:], AF.Relu, bias=B(f"gb1_{l}"))
        up = ps2.tile([C, 1024], fdt, tag="ps2")
        _mm(nc, up, W(f"gw2_{l}"), r_bf[:, sl], start=True, stop=True)
        h1 = chunk_p.tile([C, 1024], bdt, tag="h1")
        nc.scalar.activation(h1[:], up[:], AFI, bias=B(f"sgb2_{l}"), scale=S_BN)
        ap2 = ps2.tile([C, 1024], fdt, tag="ps2")
        _mm(nc, ap2, W(f"ow_{l}"), o_bf[:, sl], start=True, stop=False)
        _mm(nc, ap2, W("I2"), hB[:, sl], start=False, stop=True)
        h2 = chunk_p.tile([C, 1024], bdt, tag="h2")
        nc.scalar.activation(h2[:], ap2[:], AFI, bias=B(f"sob_{l}"), scale=S_BN)
        nc.vector.tensor_add(acc_bf[:, sl], h1[:], h2[:])

    # MLP (hidden 256 split in two bf16 halves r2a/r2b, then one accumulate)
    for ch in range(2):
        sl = slice(ch * 1024, (ch + 1) * 1024)
        r2 = []
        for mh in range(2):
            mp = ps2.tile([C, 1024], fdt, tag="ps2")
            _mm(nc, mp, W(f"m1_{l}", width=256)[:, mh * 128:(mh + 1) * 128],
                acc_bf[:, sl], start=True, stop=True)
            r2h = chunk_p.tile([C, 1024], bdt, tag=f"r2{mh}")
            bname = f"m1ba_{l}" if mh == 0 else f"m1bb_{l}"
            nc.scalar.activation(r2h[:], mp[:], AF.Relu, bias=B(bname))
            r2.append(r2h)
        m2p = ps2.tile([C, 1024], fdt, tag="ps2")
        _mm(nc, m2p, W(f"m2a_{l}"), r2[0][:], start=True, stop=False)
        _mm(nc, m2p, W(f"m2b_{l}"), r2[1][:], start=False, stop=False)
        _mm(nc, m2p, W("I1"), acc_bf[:, sl], start=False, stop=True)
        dh = chunk_p.tile([C, 1024], fdt, tag="dh")
        nc.scalar.activation(dh[:], m2p[:], AFI, bias=B(f"smb2_{l}"), scale=S_BN)
        nc.vector.tensor_add(hT[:, sl], hT[:, sl], dh[:])
        nc.vector.tensor_copy(hB[:, sl], hT[:, sl])

    if not last:
        tabs[s] = emit_table(hB)


# ---------------------------------------------------------------------------
# Entry point
# ---------------------------------------------------------------------------

_CACHE = {}


def _get_kernel(caps, offs, boffs, wcols, bcols, **kw):
    key = (tuple(caps), wcols, bcols, tuple(sorted(kw.items())))
    if key not in _CACHE:
        _CACHE[key] = build_kernel(caps, offs, boffs, wcols, bcols, **kw)
    return _CACHE[key]


def kernel(**inputs):
    in_maps, caps, offs, boffs, wcols, bcols = _pack_host(inputs)
    nc = _get_kernel(caps, offs, boffs, wcols, bcols)
    res = run_bass_kernel_spmd(nc, in_maps, core_ids=list(range(N_CORES)))
    pools = []
    for si in range(3):
        parts = [np.asarray(res.results[c]["pool_out"][si])
                 for c in range(N_CORES)]
        full = np.concatenate(parts, axis=1)          # [C, 64]
        pools.append(np.ascontiguousarray(full.T).astype(np.float32))
    return tuple(pools)
